# revision 56
# baseline (speedup 1.0000x reference)
"""Distributed causal multi-head attention block for Trainium2 (8 NeuronCores).

Problem: x[2,2048,1024] -> c_attn(QKV) -> 16-head causal attention -> c_proj.

Sharding (hardcoded): DP=2 on batch x TP=4 on heads. Core c handles
batch b=c//4 and heads 4*(c%4)..4*(c%4)+3, and computes the c_proj for
a 256-wide slice of the output-feature axis, so the host-side epilogue
is a pure concatenation.

Per core: Q^T/K^T are produced in hidden-transposed layout and V in
natural layout (with an appended ones column that makes the softmax row
sums fall out of the attention matmul), so the device graph needs zero
transposes (x arrives pre-transposed/bf16-cast from the host). Causal
attention keeps scores transposed [keys, queries]; the two heads of a
pair run concurrently in PE row-groups 0-63/64-127 writing bank-aligned
halves of one psum tile, one exp covers both, and each query tile's kt
loop is software-pipelined so the in-order PE queue never parks behind
an exp. Softmax normalization is per query tile: a single-op DVE
reciprocal, a broadcast matmul, and one fused psum-side multiply that
writes normalized z straight into this core's zg slot.

Attention runs in two token-half phases (pair 0 then pair 1 per
half). The z exchange is direct SBUF->SBUF remote DMA, not a CC
AllGather (the ncfw path costs 13-35us per op and its last op used
to eat a ~30us tail): each core broadcasts its z slice to the three
XOR-relative peers (peer = own physical tpb XOR t lands in slot t;
the host permutes each core's w_proj rows to match, using a
hardcoded logical->physical NC map that a probe kernel can
re-measure via BASS_PROBE_MAP=1). Because a data broadcast spreads
its 128 row-descriptors over all 16 SDMA engines but sems ride only
2 lanes, each phase is sealed by per-peer 16-slot sem-update
broadcasts queued behind the data on every engine ring — threshold
48 is a true arrival barrier, patched onto the first remote-chunk
c_proj matmuls after scheduling (Tile's single-core sim would
deadlock on an in-graph wait). Descriptor generation (~6us per prep
on Q7) is hoisted to kernel start on four SWDGE queues so each
phase's trigger fires the moment its z is normalized; phase (1,1)
is split per query tile to start its first half ~20us earlier.

The first execution of a fresh process both staggers core launches
by milliseconds and inherits stale semaphore values, so kernel()
runs one throwaway untraced execution first (sems are cleared at
execution teardown, and second dispatches launch in lockstep); a
tiny fire-and-forget AllGather keeps a collective in the NEFF so
the runtime still builds the global comm and syncs launches.

c_proj for token half 0 rides inside pair-1's th1 ACT-bound
attention stretches as hooks; half 1 runs two-pass at the tail:
send-independent chunks (pair 0 + own pair-1 slot) accumulate into
SBUF while the last exchange is in flight, then the three remote
pair-1 chunks finish it.

Input DMAs are issued smallest/neediest-first with x split in token
quarters, and a junk-matmul chain on the triangle mask warms the PE
clock gate while the inputs stream in.

Compute dtype bf16 on the TensorEngine, f32 scores/softmax statistics.
"""
import contextlib
import ctypes
import os
import sys
import types

import numpy as np

# ---------------------------------------------------------------- problem dims
B, S, D = 2, 2048, 1024
H, HD = 16, 64
N_CORES = 8
TP = 4                   # cores per batch group (head-parallel)
HPC = H // TP            # heads per core = 4
QCOLS = HPC * HD         # 256 q (and k, v) columns per core
ESL = D // TP            # 256 output-feature columns per core
KC = D // 128            # 8 contraction chunks
NKT = S // 128           # 16 key tiles
NQT = S // 512           # 4 query tiles (512-wide, free dim)
N_WARM = 64              # PE clock-gate warmup matmuls during input DMA


def _install_ntff_shim():
    """Make `antenv.axon_hooks` importable so BASS_TRACE profiling works."""
    if "antenv.axon_hooks" in sys.modules:
        return
    try:
        lib = ctypes.CDLL("/opt/axon/libaxon_pjrt.so")
        lib.axon_start_nrt_profile.argtypes = [ctypes.POINTER(ctypes.c_int64), ctypes.c_size_t]
        lib.axon_start_nrt_profile.restype = ctypes.c_int64
        lib.axon_stop_nrt_profile.argtypes = [ctypes.c_char_p]
        lib.axon_stop_nrt_profile.restype = ctypes.c_int64
    except (OSError, AttributeError):
        lib = None

    @contextlib.contextmanager
    def _hook(output_dir, device_ids):
        import jax
        jax.devices()
        if device_ids:
            ids = (ctypes.c_int64 * len(device_ids))(*device_ids)
            rc = lib.axon_start_nrt_profile(ids, len(device_ids))
        else:
            rc = lib.axon_start_nrt_profile(None, 0)
        if rc != 0:
            raise RuntimeError(f"axon_start_nrt_profile rc={rc}")
        try:
            yield
        finally:
            n = lib.axon_stop_nrt_profile(str(output_dir).encode())
            print(f"profile: {n} file(s) written to {output_dir}", file=sys.stderr)

    mod = types.ModuleType("antenv.axon_hooks")
    mod.get_axon_ntff_profile_hook = lambda: (_hook if lib is not None else None)
    mod.set_axon_ntff_profile_hook = lambda h: None
    sys.modules["antenv.axon_hooks"] = mod


_install_ntff_shim()

import concourse.bacc as bacc
import concourse.mybir as mybir
import concourse.tile as tile
from concourse.bass_utils import run_bass_kernel_spmd

F32 = mybir.dt.float32
BF16 = mybir.dt.bfloat16
NPBF16 = np.dtype(mybir.dt.np(BF16))
EXP = mybir.ActivationFunctionType.Exp
MUL = mybir.AluOpType.mult
ADD = mybir.AluOpType.add


def _build_probe():
    """Tiny SPMD graph: every core broadcasts its logical id to the three
    XOR-relative peers (slot t <- peer at physical tpb XOR t). Reading the
    output back gives the physical-XOR -> logical-core map, which the host
    needs to permute w_proj rows per core."""
    nc = bacc.Bacc("TRN2", target_bir_lowering=False, debug=False,
                   enable_asserts=True, num_devices=N_CORES)
    myid_d = nc.dram_tensor("myid", [128, 16], F32, kind="ExternalInput")
    ids_d = nc.dram_tensor("ids", [128, 64], F32, kind="ExternalOutput")
    with tile.TileContext(nc) as tc:
        with tc.tile_pool(name="sb", bufs=1) as sb:
            ids = sb.tile([128, 64], F32, tag="ids")
            rsem = nc.alloc_semaphore("probe_rsem")
            lsem = nc.alloc_semaphore("probe_lsem")
            # init ONLY slot 0: a local write to slots 1-3 would race the
            # peers' incoming sends (cores start with multi-us skew) and
            # clobber arrived data.
            nc.sync.dma_start(out=ids[:, 0:16], in_=myid_d[:])
            for t in (1, 2, 3):
                rdests = [None] * 8
                rdests[t] = (0, t)
                nc.gpsimd.remote_dma_broadcast(
                    out_ap=ids[:, 16 * t:16 * t + 16], in_ap=ids[:, 0:16],
                    remote_sem=rsem, local_sem=lsem, rdests=rdests)
            nc.gpsimd.trigger_dma(count=None)
            out_dma = nc.sync.dma_start(out=ids_d[:], in_=ids[:])
    # The receive wait is patched in after scheduling: Tile's single-core
    # scheduling sim can't see the cross-core sem increments and would
    # deadlock on an in-graph wait_ge.
    out_dma.wait_op(rsem, 6, "sem-ge", check=False)
    nc.compile()
    return nc


_PEER_MAP = None

# Logical core -> physical NC index, measured on this axon pod (per-core
# NTFF nd/nc idx and confirmed by the remote-dma probe). XOR-relative
# delivery only needs each TP group to sit on a 4-aligned physical set.
_PHYS = [4, 5, 6, 7, 2, 3, 0, 1]


def _default_peer_map():
    inv = {p: c for c, p in enumerate(_PHYS)}
    return [[inv[_PHYS[c] ^ t] for t in range(4)] for c in range(N_CORES)]


def _get_peer_map():
    """peer_map[c][t] = logical core whose z lands in slot t of core c
    (t=0 is self). Default comes from the hardcoded physical map; set
    BASS_PROBE_MAP=1 to measure it on the fly instead (the extra NEFF
    execution adds multi-ms core start skew to the run that follows, so
    don't do it on timed runs)."""
    global _PEER_MAP
    if _PEER_MAP is None:
        if not os.environ.get("BASS_PROBE_MAP"):
            _PEER_MAP = _default_peer_map()
            return _PEER_MAP
        nc = _build_probe()
        in_maps = [{"myid": np.full((128, 16), c, np.float32)}
                   for c in range(N_CORES)]
        res = run_bass_kernel_spmd(nc, in_maps, core_ids=list(range(N_CORES)))
        pm = []
        for c in range(N_CORES):
            ids = res.results[c]["ids"]
            row = [c] + [int(ids[0, 16 * t]) for t in (1, 2, 3)]
            assert sorted(row) == sorted(
                range(4 * (c // TP), 4 * (c // TP) + 4)
            ), f"core {c}: XOR peer set {row} escapes its TP group"
            pm.append(row)
        _PEER_MAP = pm
    return _PEER_MAP


def build_graph():
    nc = bacc.Bacc("TRN2", target_bir_lowering=False, debug=False,
                   enable_asserts=True, num_devices=N_CORES,
                   num_swdge_queues=4)

    xT_d = nc.dram_tensor("xT", [D, S], BF16, kind="ExternalInput")
    wqk_d = nc.dram_tensor("wqk", [D, 2 * QCOLS], BF16, kind="ExternalInput")
    wv_d = nc.dram_tensor("wv", [D, QCOLS], BF16, kind="ExternalInput")
    wp_d = nc.dram_tensor("wp", [D, ESL], BF16, kind="ExternalInput")
    bqk_d = nc.dram_tensor("bqk", [128, 4], F32, kind="ExternalInput")
    bv_d = nc.dram_tensor("bv", [128, QCOLS], F32, kind="ExternalInput")
    bp_d = nc.dram_tensor("bp", [128, ESL], F32, kind="ExternalInput")
    tri_d = nc.dram_tensor("tri", [128, 128], BF16, kind="ExternalInput")
    ones_d = nc.dram_tensor("ones64", [65, HD], BF16, kind="ExternalInput")
    out_d = nc.dram_tensor("out", [S, ESL], F32, kind="ExternalOutput")

    wait_patches = []
    with tile.TileContext(nc) as tc:
        with (
            tc.tile_pool(name="sb", bufs=1) as sb,
            tc.tile_pool(name="pt", bufs=6) as ptp,
            tc.tile_pool(name="ob", bufs=2) as obp,
            tc.tile_pool(name="rb", bufs=3) as rbp,
            tc.tile_pool(name="psA", bufs=2, space="PSUM") as psA,
            tc.tile_pool(name="psS", bufs=2, space="PSUM") as psS,
            tc.tile_pool(name="psZ", bufs=2, space="PSUM") as psZ,
            tc.tile_pool(name="dram", bufs=1, space="DRAM") as dram,
        ):
            # ---------------- persistent SBUF tensors ----------------
            xT_sb = sb.tile([128, KC * S], BF16, tag="xT")
            wqk_sb = sb.tile([128, KC * 2 * QCOLS], BF16, tag="wqk")
            wv_sb = sb.tile([128, KC * QCOLS], BF16, tag="wv")
            wp_sb = sb.tile([128, KC * ESL], BF16, tag="wp")
            qT_sb = sb.tile([128, 2 * S], BF16, tag="qT")
            kT_sb = sb.tile([128, 2 * S], BF16, tag="kT")
            v_sb = sb.tile([128, NKT * HPC * (HD + 1)], BF16, tag="v")
            # softmax row sums per (pair, qt): rows {0,32} pick the head
            # (matmul-legal base partitions), col slot (hp*NQT+qt)*512.
            r_sum = sb.tile([33, 2 * NQT * 512], F32, tag="rsum")
            r_inv = sb.tile([33, 2 * NQT * 512], F32, tag="rinv")
            r_invb = sb.tile([33, 2 * NQT * 512], BF16, tag="rinvb")
            z_sb = sb.tile([128, 2 * S], BF16, tag="z")
            out_acc = sb.tile([128, 8 * ESL], F32, tag="oacc")
            # zg chunk k = 4*hp + j holds pair-hp z of group sender j
            zg_sb = sb.tile([128, KC * S], BF16, tag="zg")
            bqk_sb = sb.tile([128, 4], F32, tag="bqk")
            bv_sb = sb.tile([128, QCOLS], F32, tag="bv")
            bp_sb = sb.tile([128, ESL], F32, tag="bp")
            tri_sb = sb.tile([128, 128], BF16, tag="tri")
            ones_sb = sb.tile([65, HD], BF16, tag="ones")

            # One remote sem per (pair, token-half) z-exchange phase. A data
            # broadcast spreads its 128 row-descriptors over all 16 SDMA
            # engines but carries rsem increments on only 2 lanes, so its own
            # sem says nothing about the other 14 engines' rows. Instead the
            # data sends bump a junk sem, and a follow-up 16-slot sem-update
            # broadcast per peer — queued BEHIND the data on every engine
            # ring — delivers +16 only after that engine's data drained:
            # threshold 3 peers x 16 = 48 is a true arrival barrier.
            rsems = [[nc.alloc_semaphore(f"z_rsem_{hp}_{th}")
                      for th in range(2)] for hp in range(2)]
            # phase (1,1) is split per query tile so its first half's
            # exchange starts ~20us earlier; rsem_b covers the qt3 half.
            rsem_b = nc.alloc_semaphore("z_rsem_1_1b")
            junk_rsem = nc.alloc_semaphore("z_junk_rsem")
            lsem = nc.alloc_semaphore("z_lsem")

            MM = nc.tensor.matmul

            # Fire-and-forget 0.5KB AllGather: nothing waits on it — its only
            # job is to put a collective in the NEFF so the runtime builds a
            # global comm and synchronizes the 8 device-program LAUNCHES.
            # Without one, PJRT dispatches cores up to ~5ms apart and the
            # z-exchange receive waits eat that skew on the early cores.
            sync_in = dram.tile([128, 1], F32, tag="sync_in")
            sync_out = dram.tile([N_CORES * 128, 1], F32, tag="sync_out")
            nc.gpsimd.collective_compute(
                "AllGather", mybir.AluOpType.bypass,
                replica_groups=[list(range(N_CORES))],
                ins=[sync_in.opt()], outs=[sync_out.opt()])

            # ------------- input DMAs: neediest-first, x in quarters -------------
            def load_wqk_mc(mc):
                nc.sync.dma_start(
                    out=wqk_sb[:].rearrange("p (k m c) -> p k m c",
                                            k=KC, m=4)[:, :, mc, :],
                    in_=wqk_d[:, mc * 128:(mc + 1) * 128]
                    .rearrange("(k p) c -> p k c", p=128))

            def load_x_quarter(q):
                nc.sync.dma_start(
                    out=xT_sb[:].rearrange("p (k s) -> p k s", k=KC)
                    [:, :, q * 512:(q + 1) * 512],
                    in_=xT_d[:, q * 512:(q + 1) * 512]
                    .rearrange("(k p) s -> p k s", p=128))

            nc.sync.dma_start(out=tri_sb[:], in_=tri_d[:])
            load_wqk_mc(0)                 # Q heads 0,1 — first matmuls
            load_x_quarter(0)
            nc.sync.dma_start(out=bqk_sb[:], in_=bqk_d[:])
            load_wqk_mc(2)                 # K heads 0,1
            nc.sync.dma_start(
                out=wv_sb[:].rearrange("p (k s) -> p k s", k=KC),
                in_=wv_d[:, :].rearrange("(k p) s -> p k s", p=128))
            nc.sync.dma_start(out=bv_sb[:], in_=bv_d[:])
            nc.sync.dma_start(out=ones_sb[:], in_=ones_d[:])
            load_x_quarter(1)
            load_wqk_mc(1)                 # Q heads 2,3
            load_wqk_mc(3)                 # K heads 2,3
            load_x_quarter(2)
            load_x_quarter(3)
            nc.sync.dma_start(
                out=wp_sb[:].rearrange("p (k s) -> p k s", k=KC),
                in_=wp_d[:, :].rearrange("(k p) s -> p k s", p=128))
            nc.sync.dma_start(out=bp_sb[:], in_=bp_d[:])

            # ones columns of V_aug only (col 64 of each head slot)
            nc.vector.memset(
                v_sb[:].rearrange("p (t h e) -> p t h e", t=NKT, e=HD + 1)
                [:, :, :, HD:HD + 1], 1.0)

            # ---- PE clock-gate warmup: junk matmuls on tri while inputs load ----
            warm_ps = psZ.tile([128, 512], F32, tag="z", name="warm")
            for i in range(N_WARM):
                MM(warm_ps[:, 0:128], lhsT=tri_sb[:], rhs=tri_sb[:],
                                 start=True, stop=True)

            # ---------------- projections ----------------
            def qk_proj_nt(mc, nt):   # mc 0,1 -> Q head pairs; 2,3 -> K
                ps = psA.tile([128, 512], F32, tag="m")
                for k in range(KC):
                    MM(
                        ps[:],
                        lhsT=wqk_sb[:, k * 512 + mc * 128: k * 512 + (mc + 1) * 128],
                        rhs=xT_sb[:, k * S + nt * 512: k * S + (nt + 1) * 512],
                        start=(k == 0), stop=(k == KC - 1))
                dst = qT_sb if mc < 2 else kT_sb
                c2 = mc % 2
                nc.vector.tensor_scalar_add(
                    dst[:, c2 * S + nt * 512: c2 * S + (nt + 1) * 512],
                    ps[:], bqk_sb[:, mc:mc + 1])

            def qk_proj(mc):
                for nt in range(NQT):
                    qk_proj_nt(mc, nt)

            def v_proj(t):            # V natural orientation, token tile t
                psv = psA.tile([128, QCOLS], F32, tag="m")
                for k in range(KC):
                    MM(
                        psv[:],
                        lhsT=xT_sb[:, k * S + t * 128: k * S + (t + 1) * 128],
                        rhs=wv_sb[:, k * QCOLS:(k + 1) * QCOLS],
                        start=(k == 0), stop=(k == KC - 1))
                vdst = v_sb[:].rearrange(
                    "p (t h e) -> p t h e", t=NKT, e=HD + 1)[:, t, :, 0:HD]
                nc.vector.tensor_tensor(
                    vdst,
                    psv[:].rearrange("p (h d) -> p h d", h=HPC),
                    bv_sb[:].rearrange("p (h d) -> p h d", h=HPC),
                    ADD)

            # Attention pipeline state carried ACROSS query tiles: S(next)
            # is always emitted before exp/AV(previous), so the in-order PE
            # stream never parks behind an exp — including at qt boundaries.
            # Normalization of a finished qt is deferred (norm_box) until a
            # caller-chosen point with matmuls already queued ahead of it.
            pend_box = [None]
            zaug_box = [None, None]
            norm_box = [None]

            def flush_pend():
                if pend_box[0] is None:
                    return
                hp, qt, st, pT, kt, k0, qstart, w = pend_box[0]
                pend_box[0] = None
                q0 = qt * 512
                n_kt = 4 * qt + 4
                if kt == 0:
                    # allocate this qt's accumulators here: in emission order
                    # this sits AFTER the previous qt's releasing normalize
                    # TTs, so the pool wait chain stays serviceable.
                    assert norm_box[0] is None, "normalize not emitted in time"
                    for hh_ in range(2):
                        zn = psZ.tile([HD + 1, 512], F32, tag="z",
                                      name=f"zaug{hh_}")
                        zaug_box[hh_] = zn
                ext = 512 + w
                nc.scalar.activation(pT[:, 0:ext], st[:, 0:ext], EXP, scale=0.125)
                if k0 >= q0:   # diagonal tile: causal triangle mask
                    for hh in range(2):
                        nc.vector.tensor_tensor(
                            pT[:, hh * 512: hh * 512 + 128],
                            pT[:, hh * 512: hh * 512 + 128],
                            tri_sb[:], MUL)
                vcol = kt * HPC * (HD + 1) + 2 * hp * (HD + 1)
                for hh in range(2):
                    MM(
                        zaug_box[hh][:, qstart - q0: 512],
                        lhsT=v_sb[:, vcol + hh * (HD + 1): vcol + (hh + 1) * (HD + 1)],
                        rhs=pT[:, hh * 512: hh * 512 + w],
                        start=(kt == 0), stop=(kt == n_kt - 1))
                if kt == n_kt - 1:
                    norm_box[0] = (hp, qt, zaug_box[0], zaug_box[1])
                    zaug_box[0] = zaug_box[1] = None

            def flush_norm():
                """Normalize the last finished qt: r copies, reciprocal, bf16
                cast (DVE), 1/r broadcast matmuls (PE), and one fused
                psum*psum -> SBUF multiply per head. Pair-1 qts also launch
                their AllGather here."""
                if norm_box[0] is None:
                    return
                hp, qt, za0, za1 = norm_box[0]
                norm_box[0] = None
                zas = (za0, za1)
                blk = (hp * NQT + qt) * 512
                for hh in range(2):
                    nc.vector.tensor_copy(
                        r_sum[32 * hh:32 * hh + 1, blk:blk + 512],
                        zas[hh][HD:HD + 1, :])
                nc.vector.reciprocal_approx_fast(
                    out=r_inv[:, blk:blk + 512], in_=r_sum[:, blk:blk + 512])
                nc.vector.tensor_copy(r_invb[:, blk:blk + 512],
                                      r_inv[:, blk:blk + 512])
                for hh in range(2):
                    ro = 32 * hh
                    rbc = psA.tile([HD, 512], F32, tag="m", name=f"rbc{hh}")
                    MM(rbc[:], lhsT=ones_sb[ro:ro + 1, :],
                                     rhs=r_invb[ro:ro + 1, blk:blk + 512],
                                     start=True, stop=True)
                    rbc_sb = rbp.tile([HD, 512], F32, tag="rb")
                    nc.vector.tensor_copy(rbc_sb[:], rbc[:])
                    ho = hh * HD
                    nc.vector.tensor_tensor(
                        z_sb[ho:ho + HD,
                             hp * S + qt * 512: hp * S + (qt + 1) * 512],
                        zas[hh][0:HD, :], rbc_sb[:], MUL)
                if hp == 1 and qt == 2:
                    send_hp_th(1, 1)          # (1,1) qt2 half
                elif qt % 2 == 1:
                    if hp == 1 and qt == 3:
                        send_11b()            # (1,1) qt3 half
                    else:
                        send_hp_th(hp, qt // 2)

            def attention_qt(hp, qt, post_kt=None):
                # Both heads of the pair run together: their K=64 score
                # matmuls occupy PE row-groups 0-63 / 64-127 concurrently and
                # write the two bank-aligned halves of one psum tile, so one
                # exp call covers both heads. post_kt: {kt: fn} extra emission
                # hooks (c_proj quarters riding the ACT-bound stretches).
                q0 = qt * 512
                for kt in range(4 * qt + 4):
                    k0 = kt * 128
                    qstart = max(q0, k0)
                    w = q0 + 512 - qstart
                    st = psS.tile([128, 1024], F32, tag="s")
                    pT = ptp.tile([128, 1024], BF16, tag="pT")
                    for hh in range(2):
                        ho = hh * HD
                        MM(
                            st[:, hh * 512: hh * 512 + w],
                            lhsT=kT_sb[ho:ho + HD, hp * S + k0: hp * S + k0 + 128],
                            rhs=qT_sb[ho:ho + HD, hp * S + qstart: hp * S + qstart + w],
                            start=True, stop=True)
                    if kt == 1:
                        flush_norm()   # previous qt's normalize, 2 S-pairs deep
                    flush_pend()
                    pend_box[0] = (hp, qt, st, pT, kt, k0, qstart, w)
                    if post_kt is not None and kt in post_kt:
                        post_kt[kt]()

            # ------- z exchange: direct SBUF->SBUF sends to the 3 peers -------
            # Peer (own physical tpb XOR t) stores our data at its slot t, so
            # one shared out_ap per t works on every core (XOR is symmetric:
            # my slot t then holds that same peer's z). Slot 0 is self — c_proj
            # reads it straight out of z_sb, no copy. Each send rides 2 SDMA
            # lanes (~5us for 0.25MB), the three sends of a phase ride
            # disjoint lanes concurrently, and the host permutes each core's
            # w_proj rows per the probed XOR->logical map, so the epilogue
            # stays a pure concatenation.
            # Q7 descriptor generation is slow (~6us per 128-row prep), so
            # all four phases' descriptors are generated up front on their
            # own SWDGE queues while the inputs stream in; each phase's
            # trigger then fires instantly once its z is written (the
            # trigger carries the deferred z_sb read deps).
            def emit_send_preps(hp, th, q, c0, c1, sem):
                src = z_sb[:, hp * S + th * 1024 + c0:
                           hp * S + th * 1024 + c1]
                for t in (1, 2, 3):
                    k = TP * hp + t
                    rdests = [None] * 8
                    rdests[t] = (0, t)
                    nc.gpsimd.remote_dma_broadcast(
                        out_ap=zg_sb[:, k * S + th * 1024 + c0:
                                     k * S + th * 1024 + c1],
                        in_ap=src,
                        remote_sem=junk_rsem, local_sem=lsem,
                        rdests=rdests, queue_num=q)
                for t in (1, 2, 3):
                    nc.gpsimd.remote_sem_update_broadcast(
                        remote_sem=sem, local_sem=lsem,
                        rdests=[(0, t)] * 16, queue_num=q)

            def send_hp_th(hp, th):
                # phases fire on queues 0..3; (1,1) fires only its qt2 half
                # here (queue 3) — the qt3 half rides the re-used queue 0.
                # Its preps are emitted at the (0,1) trigger: late enough
                # that no other phase's trigger queues behind their ~19us
                # of Q7 descgen, early enough to be ready for the last send.
                nc.gpsimd.trigger_dma(count=None, queue_num=2 * hp + th)
                if (hp, th) == (0, 1):
                    emit_send_preps(1, 1, 0, 512, 1024, rsem_b)

            def send_11b():
                nc.gpsimd.trigger_dma(count=None, queue_num=0)

            # pre-generate the phases' send descriptors now (gpsimd is
            # idle from t~0, so Q7 churns through the preps during the
            # input-DMA / early-attention window, well before any trigger)
            emit_send_preps(0, 0, 0, 0, 1024, rsems[0][0])
            emit_send_preps(1, 0, 2, 0, 1024, rsems[1][0])
            emit_send_preps(0, 1, 1, 0, 1024, rsems[0][1])
            emit_send_preps(1, 1, 3, 0, 512, rsems[1][1])

            def zg_chunk(k, mt):
                # c_proj lhsT for contraction chunk k, token tile mt:
                # slot 0 of each pair is this core's own z.
                hp_k, t_k = k // TP, k % TP
                if t_k == 0:
                    return z_sb[:, hp_k * S + mt * 128: hp_k * S + (mt + 1) * 128]
                return zg_sb[:, k * S + mt * 128: k * S + (mt + 1) * 128]

            # ------------- c_proj: single pass per token quarter -------------
            # The MM that reads the FIRST remote chunk of each (pair, half)
            # within an accumulation group gets a receive wait patched on
            # after scheduling (Tile's single-core sim can't see cross-core
            # sem increments and would deadlock on an in-graph wait_ge);
            # later chunks of the pair follow it in tensor-queue order.
            def cp_mm(po, k, mt, tq, start, stop):
                inst = MM(po[:], lhsT=zg_chunk(k, mt),
                          rhs=wp_sb[:, k * ESL:(k + 1) * ESL],
                          start=start, stop=stop)
                if k % TP == 1:
                    wait_patches.append((inst, k // TP, tq // 2, tq))

            # each quarter's groups are emitted one at a time so they can be
            # hooked into attention kt slots (filling ACT-bound PE idle);
            # state[0] holds the quarter's o_sb across groups.
            q_state = {}

            def c_proj_group(tq, i):
                if i == 0:
                    q_state[tq] = obp.tile([128, NQT * ESL], F32, tag="o",
                                           name=f"oq{tq}")
                o_sb = q_state[tq]
                mt = 4 * tq + i
                po = psA.tile([128, ESL], F32, tag="m")
                for k in range(KC):
                    cp_mm(po, k, mt, tq, k == 0, k == KC - 1)
                nc.vector.tensor_tensor(o_sb[:, i * ESL:(i + 1) * ESL],
                                        po[:], bp_sb[:], ADD)
                if i == 3:
                    nc.sync.dma_start(
                        out=out_d[tq * 512:(tq + 1) * 512, :]
                        .rearrange("(m p) c -> p m c", p=128),
                        in_=o_sb[:].rearrange("p (m c) -> p m c", m=NQT))

            def c_proj_quarter(tq):
                for i in range(4):
                    c_proj_group(tq, i)

            KP1 = 5   # pass1 covers pair-0 chunks + pair-1's own (slot 0)

            def c_proj_pass1(tq):
                # send-independent chunks -> SBUF accumulator; runs while
                # the last (pair-1 qt3) z exchange is still in flight.
                for i in range(4):
                    mt = 4 * tq + i
                    po = psA.tile([128, ESL], F32, tag="m")
                    for k in range(KP1):
                        cp_mm(po, k, mt, tq, k == 0, k == KP1 - 1)
                    nc.vector.tensor_tensor(
                        out_acc[:, (mt - 8) * ESL:(mt - 7) * ESL],
                        po[:], bp_sb[:], ADD)

            def c_proj_pass2(tq):
                o_sb = obp.tile([128, NQT * ESL], F32, tag="o")
                for i in range(4):
                    mt = 4 * tq + i
                    po = psA.tile([128, ESL], F32, tag="m")
                    for k in range(KP1, KC):
                        cp_mm(po, k, mt, tq, k == KP1, k == KC - 1)
                    nc.vector.tensor_tensor(
                        o_sb[:, i * ESL:(i + 1) * ESL], po[:],
                        out_acc[:, (mt - 8) * ESL:(mt - 7) * ESL], ADD)
                nc.sync.dma_start(
                    out=out_d[tq * 512:(tq + 1) * 512, :]
                    .rearrange("(m p) c -> p m c", p=128),
                    in_=o_sb[:].rearrange("p (m c) -> p m c", m=NQT))

            # -------- two token-half phases: pair 0 then pair 1 per half ----
            # Projections trickle per query tile: attention(0, qt) needs only
            # Q tile qt, K tiles <= qt and V tokens < (qt+1)*512, so the first
            # exps start as soon as the nt=0 projections land. The half-0
            # gather fires mid-kernel, so the two collectives never queue on
            # the serial ncfw engine.
            for th in range(2):
                for qt in (2 * th, 2 * th + 1):
                    if qt != 2:
                        # tile-2 Q/K projections ride attention(1,1) hooks
                        qk_proj_nt(0, qt)     # Q heads 0,1 tile qt
                        qk_proj_nt(2, qt)     # K heads 0,1 tile qt
                    if th == 0:
                        for tt in range(4 * qt, 4 * qt + 4):
                            v_proj(tt)
                    hooks = {}
                    if qt == 1:
                        # pair-1 Q/K projections ride pair-0's attention so
                        # their DVE bias writes land well before pair-1's
                        # score matmuls need them (kills the transition gap).
                        hooks = {1: lambda: qk_proj_nt(1, 0),
                                 3: lambda: qk_proj_nt(1, 1),
                                 5: lambda: qk_proj_nt(3, 0),
                                 7: lambda: qk_proj_nt(3, 1)}
                    elif qt == 2:
                        # V tiles 12-15 (first needed by attention(0,3))
                        # fill qt2's ACT-bound stretches.
                        hooks = {1: lambda: v_proj(12), 4: lambda: v_proj(13),
                                 7: lambda: v_proj(14), 10: lambda: v_proj(15)}
                    elif qt == 3:
                        hooks = {2: lambda: qk_proj_nt(1, 2),
                                 5: lambda: qk_proj_nt(1, 3),
                                 8: lambda: qk_proj_nt(3, 2),
                                 11: lambda: qk_proj_nt(3, 3)}
                    attention_qt(0, qt, post_kt=hooks)
                flush_pend()
                for qt in (2 * th, 2 * th + 1):
                    hooks = {}
                    if th == 0 and qt == 0:
                        # V tiles 8-11 (needed only from attention(0,2) in
                        # th1) ride the ACT-bound stretches here, filling
                        # the phase-transition PE gaps.
                        hooks = {1: lambda: v_proj(8), 3: lambda: v_proj(9)}
                    if th == 0 and qt == 1:
                        # tile-2 Q/K projections land here so attention(0,2)
                        # starts the instant th1 begins.
                        hooks = {2: lambda: v_proj(10),
                                 3: lambda: qk_proj_nt(0, 2),
                                 5: lambda: v_proj(11),
                                 6: lambda: qk_proj_nt(2, 2)}
                    if th == 1 and qt == 2:
                        # half-0 c_proj rides pair-1's attention: its z
                        # chunks landed ~40us ago (sems 48 by ~110-130us).
                        hooks = {1: lambda: c_proj_group(0, 0),
                                 4: lambda: c_proj_group(0, 1),
                                 7: lambda: c_proj_group(0, 2),
                                 10: lambda: c_proj_group(0, 3)}
                    if th == 1 and qt == 3:
                        hooks = {1: lambda: c_proj_group(1, 0),
                                 4: lambda: c_proj_group(1, 1),
                                 7: lambda: c_proj_group(1, 2),
                                 10: lambda: c_proj_group(1, 3)}
                    attention_qt(1, qt, post_kt=hooks)
                flush_pend()
                flush_norm()                  # normalize (1, 2*th+1) + send
            # the tail: only the half-1 c_proj remains; pass1 (pair-0
            # chunks) fills the final z-exchange's flight, pass2 follows.
            c_proj_pass1(2)
            c_proj_pass1(3)
            c_proj_pass2(2)
            c_proj_pass2(3)

    for inst, hp, th, tq in wait_patches:
        sem = rsem_b if (hp == 1 and th == 1 and tq == 3) else rsems[hp][th]
        inst.wait_op(sem, 48, "sem-ge", check=False)
    nc.compile()
    return nc


_NC = None


def _get_nc():
    global _NC
    if _NC is None:
        _NC = build_graph()
    return _NC


def _make_in_maps(x, w_attn, b_attn, w_proj, b_proj):
    x = np.asarray(x, dtype=np.float32)
    w_attn = np.asarray(w_attn, dtype=np.float32)
    b_attn = np.asarray(b_attn, dtype=np.float32)
    w_proj = np.asarray(w_proj, dtype=np.float32)
    b_proj = np.asarray(b_proj, dtype=np.float32)

    tri = np.triu(np.ones((128, 128), np.float32)).astype(NPBF16)  # tri[k,j]=1 iff j>=k
    ones64 = np.ones((65, HD), np.float32).astype(NPBF16)
    xT = [np.ascontiguousarray(x[b].T).astype(NPBF16) for b in range(B)]

    peer_map = _get_peer_map()
    in_maps = []
    for c in range(N_CORES):
        b, hg = c // TP, c % TP
        qs, ks, vs = hg * QCOLS, D + hg * QCOLS, 2 * D + hg * QCOLS
        es = (c % TP) * ESL
        wqk = np.concatenate(
            [w_attn[:, qs:qs + QCOLS], w_attn[:, ks:ks + QCOLS]], axis=1
        ).astype(NPBF16)
        wv = np.ascontiguousarray(w_attn[:, vs:vs + QCOLS]).astype(NPBF16)
        # zg chunk k=4*hp+t holds pair hp of the XOR-t peer (t=0 is self);
        # permute w_proj rows to match this core's received layout.
        perm = np.empty(D, np.int64)
        for k in range(KC):
            hp_, t = k // TP, k % TP
            j = peer_map[c][t] % TP
            for p in range(128):
                perm[k * 128 + p] = (4 * j + 2 * hp_ + p // HD) * HD + p % HD
        wp = np.ascontiguousarray(w_proj[perm][:, es:es + ESL]).astype(NPBF16)
        bqk = np.stack([b_attn[qs:qs + 128], b_attn[qs + 128:qs + QCOLS],
                        b_attn[ks:ks + 128], b_attn[ks + 128:ks + QCOLS]],
                       axis=1).astype(np.float32)
        bv = np.ascontiguousarray(
            np.broadcast_to(b_attn[vs:vs + QCOLS], (128, QCOLS))).astype(np.float32)
        bp = np.ascontiguousarray(
            np.broadcast_to(b_proj[es:es + ESL], (128, ESL))).astype(np.float32)
        in_maps.append({
            "xT": xT[b], "wqk": wqk, "wv": wv, "wp": wp,
            "bqk": bqk, "bv": bv, "bp": bp, "tri": tri, "ones64": ones64,
        })
    return in_maps


_WARMED = False


def kernel(x, w_attn, b_attn, w_proj, b_proj):
    global _WARMED
    nc = _get_nc()
    in_maps = _make_in_maps(x, w_attn, b_attn, w_proj, b_proj)
    if not _WARMED:
        # The first dispatch of an executable staggers core launches by
        # milliseconds AND the z-exchange semaphores hold stale values from
        # whatever ran before (they are cleared only at execution teardown).
        # One throwaway untraced execution fixes both; its output is junk.
        os.environ["BASS_NEVER_TRACE"] = "1"
        try:
            run_bass_kernel_spmd(nc, in_maps, core_ids=list(range(N_CORES)),
                                 trace=False)
        finally:
            os.environ.pop("BASS_NEVER_TRACE", None)
        _WARMED = True
    res = run_bass_kernel_spmd(nc, in_maps, core_ids=list(range(N_CORES)),
                               trace=bool(os.environ.get("BASS_TRACE")))
    if res.exec_time_ns is not None:
        print(f"HW exec time: {res.exec_time_ns} ns")
    out = np.empty((B, S, D), np.float32)
    for c in range(N_CORES):
        b, es = c // TP, (c % TP) * ESL
        out[b, :, es:es + ESL] = res.results[c]["out"]
    return out

